# revision 56
# baseline (speedup 1.0000x reference)
"""Trainium2 Bass kernel for nn_CGSC_64914135712264.

Sharding: 8 cores = (batch b in 0..3) x (H-half in 0..1). Each core computes
a [C=128, 28, 56] output slab of its batch. All params replicated.

v2 design (engine-balanced, 2.8x faster than v1 in the TimelineSim cost
model: 266us -> 95us):
  - v/q/fus 1x1 convs are folded away on the host: the dynamic-conv broadcast
    weights are channel-uniform, so bc*(Wv x) = Wv(bc*x); the per-tap PSUM
    accumulation uses lhsT=(fus@Wv)^T, and scores use M=(key^T Wq) directly
    (no q tensor). The v-bias term is a rank-1 matmul fb x colsum(dynw).
  - Per tap: broadcast dynw row (DMA broadcast from a DRAM round-trip, or
    selector-matmul + Act copies), bf16 DVE multiply with the raw x window
    (2x DVE mode), PE accumulating matmuls into 4 chunk PSUM banks.
  - lepe 7x7 reparam conv split: PE diag matmuls into the same banks, plus a
    DVE product/add tree (run before the mul pipeline) merged via one
    identity matmul.
  - Some early dyn-conv multiplies run on the (otherwise idle) Pool engine;
    their accumulations are deferred to late bank order so the strictly
    ordered PSUM accumulate chain never waits on a slow Pool mul.
  - Channel attention sums come free via Act accum_out; pair AllReduce for
    cross-half stats; sigmoid via tanh (keeps Act at 2 table loads).
  - Constants ride in per-dtype blob DMAs; output is bf16 (host upcasts).
"""

import sys

sys.path.insert(0, "/opt/trn_rl_repo")

import numpy as np
import ml_dtypes

import concourse.bass as bass
import concourse.mybir as mybir
import concourse.tile as tile
from concourse import bacc
from concourse.ap import AP as BassAP
from concourse.bass_utils import run_bass_kernel_spmd

BF16 = ml_dtypes.bfloat16
FP8 = ml_dtypes.float8_e4m3
F32 = mybir.dt.float32
BF = mybir.dt.bfloat16
F8 = mybir.dt.float8e4

B, C, H, W = 4, 128, 56, 56
K = 7
K2 = 49
HALF = 28          # rows per core
HP, WP = HALF + 6, W + 6   # padded tile 34 x 62
N = HALF * W       # 1568 free elems per core
CH_N = 7 * W       # 392 per chunk
NCHUNK = 4
SCALE = float(C) ** -0.5

# ---- tap assignment (tunable) ----
# lepe: all 49 taps via fp8e4m3 DoubleRow paired diag matmuls (2 taps per
# matmul at 0.5 cyc/col) on an fp8 copy of the padded x tile. Pairs must have
# non-increasing AP strides, so taps are paired VERTICALLY (same kw, kh and
# kh+1: window offset delta = WP) plus a kh=6 single per column (delta=0
# dummy second window against a zeroed weight tile).
# Entries: (kh_a, kw, kh_b or None)
LEPE_PAIRS = [(kh, kw, kh + 1 if kh < 6 else None)
              for kw in range(K) for kh in (0, 2, 4, 6)]
MUL_POOL = [1, 4, 6, 9, 13, 16, 21, 25]  # Pool muls: early taps only, so
                                       # their bcasts arrive early on the wall
# each Pool-mul tap's accumulation is injected back into the PE stream this
# many taps later (the Pool mul is done by then), instead of as one serial
# batch at the end
POOL_INJECT = 10
# bcast path per tap: fp8-DoubleRow selector-matmul + Act copies, or DMA
# broadcast from DRAM. Act-path taps are spread 1-in-3 mid-stream and cover
# the tail densely: the serialized DMA broadcast chain then ends several taps
# before the stream does, so the tail is never DMA-supply-starved.
BCAST_ACT = ({k for k in range(K2) if k % 3 == 2} |
             {41, 43, 44, 45, 46, 47, 48})
_CACHE = {}


def _build_program(use_collectives=True):
    nc = bacc.Bacc("TRN2", target_bir_lowering=False, debug=False, num_devices=8)
    f32, bf = F32, BF

    # ---- DRAM I/O (constants packed into per-dtype blobs: few big DMAs
    # instead of ~24 small ones serializing on the HWDGE) ----
    n_act = len(BCAST_ACT)
    d_xpad = nc.dram_tensor("x_pad", [C, HP, WP], bf, kind="ExternalInput")
    d_xid = nc.dram_tensor("x_id", [C, N], bf, kind="ExternalInput")
    d_xup = nc.dram_tensor("x_up", [C, H, W], bf, kind="ExternalInput")
    d_xpad8 = nc.dram_tensor("x_pad8", [C, HP, WP], F8, kind="ExternalInput")
    d_lw8 = nc.dram_tensor("lepe_w8", [C, len(LEPE_PAIRS), 2, C], F8,
                           kind="ExternalInput")
    # blob_bf [C, 3*128+1]: wgT m0T fwvT mtb | blob_f32 [C, 12]:
    # caw1T(8) ebv bf gbs gbb
    d_blob_bf = nc.dram_tensor("blob_bf", [C, 3 * C + 2], bf, kind="ExternalInput")
    d_blob_f32 = nc.dram_tensor("blob_f32", [C, 12], f32, kind="ExternalInput")
    # blob49 [49, 51]: wwT(49) ones49(1) ebk(1) | blob1 [1, 226]
    d_blob49 = nc.dram_tensor("blob49", [K2, 51], bf, kind="ExternalInput")
    # fp8 selector stack for DoubleRow broadcast matmuls (2nd tile zeroed)
    d_est8 = nc.dram_tensor("est8", [K2, n_act, 2, C], F8, kind="ExternalInput")
    d_blob1 = nc.dram_tensor("blob1", [1, 2 * K2 + C], bf, kind="ExternalInput")
    d_caw2T = nc.dram_tensor("ca_w2T", [8, C], f32, kind="ExternalInput")
    d_y = nc.dram_tensor("y", [C, N], bf, kind="ExternalOutput")

    # collectives scratch (pair allreduce for channel attention)
    cc_in_s = nc.dram_tensor("cc_in_s", [C, 1], f32)
    cc_in_m = nc.dram_tensor("cc_in_m", [C, 1], f32)
    cc_out_s = nc.dram_tensor("cc_out_s", [C, 1], f32)
    cc_out_m = nc.dram_tensor("cc_out_m", [C, 1], f32)
    groups = [[0, 1], [2, 3], [4, 5], [6, 7]]

    AF = mybir.ActivationFunctionType
    ALU = mybir.AluOpType
    AX = mybir.AxisListType

    with tile.TileContext(nc, trace_sim=False) as tc:
        with (
            tc.tile_pool(name="const", bufs=1) as constp,
            tc.tile_pool(name="big", bufs=1) as bigp,
            tc.tile_pool(name="work", bufs=2) as workp,
            tc.tile_pool(name="dram", bufs=1, space="DRAM") as dramp,
        ):
            def load(pool, dram, shape, dtype):
                t = pool.tile(shape, dtype, tag=dram.name)
                nc.sync.dma_start(out=t[:], in_=dram[:])
                return t

            # input order: x_up first (the pooling -> key -> scores -> dynw
            # chain is the program's critical path), then xpad/weights
            # act-table preload: a dummy Exp on a memset tile triggers the
            # LoadActFuncSet at t~0, overlapping the input DMAs instead of
            # sitting on the attention critical path
            dumm = constp.tile([1, 2], f32, tag="dumm")
            nc.vector.memset(dumm[:, 0:1], 0.0)
            nc.scalar.activation(dumm[:, 1:2], dumm[:, 0:1],
                                 AF.Exp, bias=0.0, scale=1.0)

            xup = bigp.tile([C, H, W], bf, tag="x_up")
            nc.sync.dma_start(out=xup[:, 0:28, :], in_=d_xup[:, 0:28, :])
            nc.sync.dma_start(out=xup[:, 28:, :], in_=d_xup[:, 28:, :])
            blob_bf = load(constp, d_blob_bf, [C, 3 * C + 2], bf)
            blob_f32 = load(constp, d_blob_f32, [C, 12], f32)
            xpad = load(bigp, d_xpad, [C, HP, WP], bf)
            blob49 = load(bigp, d_blob49, [K2, 51], bf)
            blob1 = load(constp, d_blob1, [1, 2 * K2 + C], bf)
            caw2T = load(constp, d_caw2T, [8, C], f32)
            xpad8 = load(bigp, d_xpad8, [C, HP, WP], F8)
            lw8 = load(constp, d_lw8, [C, len(LEPE_PAIRS), 2, C], F8)
            est8 = load(constp, d_est8, [K2, n_act, 2, C], F8)
            xid = load(bigp, d_xid, [C, N], bf)

            wgT = blob_bf[:, 0:C]
            m0T = blob_bf[:, C:2 * C]
            fwvT = blob_bf[:, 2 * C:3 * C]
            mtb = blob_bf[:, 3 * C:3 * C + 1]
            ebv = blob_bf[:, 3 * C + 1:3 * C + 2]
            caw1T = blob_f32[:, 0:8]
            bfcol = blob_f32[:, 9:10]
            gbs = blob_f32[:, 10:11]
            gbb = blob_f32[:, 11:12]
            wwT = blob49[:, 0:K2]
            ones49 = blob49[:, K2:K2 + 1]
            ebk = blob49[:, K2 + 1:K2 + 2]
            ones1_49 = blob1[:, 0:K2]
            bwrow = blob1[:, K2:2 * K2]
            fbrow = blob1[:, 2 * K2:2 * K2 + C]

            def win(src, k, nch=NCHUNK):
                kh, kw = divmod(k, K)
                return src[:, kh:kh + nch * 7, kw:kw + W].rearrange(
                    "p (a r) w -> p a r w", a=nch)

            def chunks4(t):
                return t[:].rearrange("p (a r w) -> p a r w", a=NCHUNK, r=7, w=W)

            # ---- lepe DoubleRow tap-pair helper: one fp8 matmul = 2 taps.
            # Vertical pairs: windows at (kh, kw) and (kh+1, kw), pair-dim
            # stride WP; kh=6 singles use a delta-0 dummy second window with
            # zeroed weights.
            def lepe_rhs(kh_a, kw, kh_b, ci):
                off_a = (kh_a + ci * 7) * WP + kw
                dlt = 0 if kh_b is None else (kh_b - kh_a) * WP
                return BassAP(xpad8[:].tensor, off_a,
                              [[HP * WP, C], [dlt, 2], [WP, 7], [1, W]])

            # ============ phase A: gate, pooled key, attention ============
            gate = bigp.tile([C, N], bf)
            expv = bigp.tile([49, N], bf)
            zrow = bigp.tile([1, N], bf)
            dynw = bigp.tile([K2, N], bf)
            dynw8 = bigp.tile([K2, N], F8)
            recbc = bigp.tile([K2, N], f32)
            srow = bigp.tile([1, N], bf)
            ddyn = dramp.tile([K2, N], BF, tag="ddyn")

            # accumulation banks claimed FIRST so they don't alias phase-A
            # PSUM: lepe matmuls can then start during the attention phase
            accp_ctx = tc.tile_pool(name="acc", bufs=1, space="PSUM")
            accp = accp_ctx.__enter__()
            acc = accp.tile([C, NCHUNK, 512], f32, tag="acc")

            with (
                tc.tile_pool(name="psA", bufs=4, space="PSUM") as psA,
                tc.high_priority(),
            ):
                # x_up [C, 56, 56] -> block-sum 8x8 -> [C, 7, 7] (sums; /64 in
                # wk); two halves pipelined with the two x_up DMAs
                pool1 = workp.tile([C, H, 7], f32, tag="pool1")
                for hh in range(2):
                    nc.vector.tensor_reduce(
                        out=pool1[:, hh * 28:(hh + 1) * 28, :],
                        in_=xup[:, hh * 28:(hh + 1) * 28, :].rearrange(
                            "p h (bw dw) -> p h bw dw", dw=8),
                        axis=AX.X, op=ALU.add)
                pooled = workp.tile([C, K2], bf, tag="pooled")
                with nc.allow_low_precision(reason="8-term block sum to bf16"):
                    nc.vector.tensor_reduce(
                        out=pooled[:].rearrange("p (a b) -> p a b", a=7),
                        in_=pool1[:].rearrange("p (bh dh) bw -> p bh bw dh", dh=8),
                        axis=AX.X, op=ALU.add)

                # M^T = (Wq^T Wk') pooled + Wq^T bk  (key matmul folded into
                # mt on the host: one matmul instead of two chained ones on
                # the attention critical path)
                mt_ps = psA.tile([C, 512], f32, tag="big49")
                nc.tensor.matmul(mt_ps[:, :K2], lhsT=m0T, rhs=pooled[:],
                                 start=True, stop=True)
                mt = workp.tile([C, K2], bf, tag="mt")
                nc.scalar.activation(mt[:], mt_ps[:, :K2], AF.Identity,
                                     bias=mtb, scale=1.0)
                # exp bias eb = SCALE*(key^T bq) = SCALE*pooled^T(Wk'^T bq)
                # + SCALE*(bk.bq)  (second term host-folded into ebk column)
                eb_ps = psA.tile([K2, 512], f32, tag="big49")
                nc.tensor.matmul(eb_ps[:, :1], lhsT=pooled[:], rhs=ebv,
                                 start=True, stop=True)
                eb = workp.tile([K2, 1], f32, tag="eb")
                nc.vector.scalar_tensor_tensor(eb[:], eb_ps[:, :1], SCALE,
                                               ebk, ALU.mult, ALU.add)

                # scores -> exp -> Z -> 1/Z bcast -> dU -> dynw, pipelined
                # chunk-major through 2 rotating single-bank buffers so the
                # normalization never waits on a later chunk's scores
                for ci in range(NCHUNK):
                    sl = slice(ci * CH_N, (ci + 1) * CH_N)
                    sc_ps = psA.tile([K2, 512], f32, tag="big49")
                    nc.tensor.matmul(
                        sc_ps[:, :CH_N], lhsT=mt[:],
                        rhs=xpad[:, 3 + ci * 7:10 + ci * 7, 3:3 + W],
                        start=True, stop=True)
                    nc.scalar.activation(expv[:, sl], sc_ps[:, :CH_N],
                                         AF.Exp, bias=eb[:], scale=SCALE)
                    z_ps = psA.tile([1, 512], f32, tag="big49")
                    nc.tensor.matmul(z_ps[:, :CH_N], lhsT=ones49,
                                     rhs=expv[:, sl], start=True, stop=True)
                    nc.scalar.activation(zrow[:, sl], z_ps[:, :CH_N],
                                         AF.Identity, bias=0.0, scale=1.0)
                    zbc_ps = psA.tile([K2, 512], f32, tag="big49")
                    nc.tensor.matmul(zbc_ps[:, :CH_N], lhsT=ones1_49,
                                     rhs=zrow[:, sl], start=True, stop=True)
                    nc.vector.reciprocal(recbc[:, sl], zbc_ps[:, :CH_N])
                    dU_ps = psA.tile([K2, 512], f32, tag="big49")
                    nc.tensor.matmul(dU_ps[:, :CH_N], lhsT=wwT,
                                     rhs=expv[:, sl], start=True, stop=False)
                    nc.tensor.matmul(dU_ps[:, :CH_N], lhsT=bwrow,
                                     rhs=zrow[:, sl], start=False, stop=True)
                    nc.vector.tensor_mul(dynw[:, sl], dU_ps[:, :CH_N],
                                         recbc[:, sl])

                # dynw -> DRAM for broadcast reads
                nc.sync.dma_start(out=ddyn[:], in_=dynw[:])

            # fp8 dynw copy for the DoubleRow selector bcasts -- outside the
            # high-priority block so the casts never displace phase-A exps
            for ci in range(NCHUNK):
                sl = slice(ci * CH_N, (ci + 1) * CH_N)
                nc.scalar.activation(dynw8[:, sl], dynw[:, sl],
                                     AF.Identity, bias=0.0, scale=1.0)

            # ============ phase B: dynamic conv + lepe accumulation ============
            xf = bigp.tile([C, N], bf)
            asum = workp.tile([C, NCHUNK], f32, tag="asum")
            with (
                tc.tile_pool(name="bcps", bufs=2, space="PSUM") as bcpsp,
                tc.tile_pool(name="bcs", bufs=10) as bcsp,
                tc.tile_pool(name="bca", bufs=8) as bcap,
                tc.tile_pool(name="prods", bufs=8) as prodsp,
            ):
                started = [False] * NCHUNK
                n_acc = [0] * NCHUNK
                # total accumulating matmuls per bank:
                per_bank = K2 + len(LEPE_PAIRS) + 1

                def accum(ci, lhsT, rhs, **kw):
                    nc.tensor.matmul(acc[:, ci, :CH_N], lhsT=lhsT, rhs=rhs,
                                     start=not started[ci],
                                     stop=n_acc[ci] == per_bank - 1, **kw)
                    started[ci] = True
                    n_acc[ci] += 1

                # Act-path broadcast producer: fp8 DoubleRow selector matmul
                # into PSUM + two Act copies to bf16 SBUF. Tiles rotate
                # through a lookahead pool so production runs ahead of the
                # tap stream without pinning one tile per tap.
                act_idx = {k: j for j, k in enumerate(sorted(BCAST_ACT))}

                def act_bcast(k):
                    bct = bcap.tile([C, N], bf, tag="bca")
                    bc_ps = bcpsp.tile([C, 2, 512], f32, tag="bcps")
                    for half in range(2):
                        for cj in range(2):
                            ci = half * 2 + cj
                            rhs8 = BassAP(dynw8[:].tensor, ci * CH_N,
                                          [[N, K2], [0, 2], [1, CH_N]])
                            nc.tensor.matmul(
                                bc_ps[:, cj, :CH_N],
                                lhsT=est8[:, act_idx[k], :, :], rhs=rhs8,
                                start=True, stop=True,
                                perf_mode=mybir.MatmulPerfMode.DoubleRow)
                        nc.scalar.activation(
                            bct[:, half * 784:(half + 1) * 784].rearrange(
                                "p (a x) -> p a x", a=2),
                            bc_ps[:, :, :CH_N], AF.Identity, bias=0.0, scale=1.0)
                    return bct

                # lepe tap-pairs next in bank order: fp8 DoubleRow matmuls
                # that only need xpad8+lw8, so the PE chews through them
                # during the attention phase (acc banks are disjoint from
                # phase-A PSUM)
                for j, (kh_a, kw, kh_b) in enumerate(LEPE_PAIRS):
                    for ci in range(NCHUNK):
                        accum(ci, lw8[:, j, :, :], lepe_rhs(kh_a, kw, kh_b, ci),
                              perf_mode=mybir.MatmulPerfMode.DoubleRow)

                # per-bank epilogue: emit xf (fused bias + Act accum_out row
                # sums) the moment a bank's accumulation stops; Pool takes the
                # max-reduce and DVE the gate-mul so the three chase each
                # bank while later banks still accumulate
                amax = workp.tile([C, NCHUNK], f32, tag="amax")
                t1 = bigp.tile([C, N], bf)

                def emit_bank(ci):
                    sl = slice(ci * CH_N, (ci + 1) * CH_N)
                    nc.scalar.activation(
                        xf[:, sl], acc[:, ci, :CH_N],
                        AF.Identity, bias=bfcol, scale=1.0,
                        accum_out=asum[:, ci:ci + 1])
                    nc.vector.tensor_reduce(out=amax[:, ci:ci + 1],
                                            in_=xf[:, sl], axis=AX.X,
                                            op=ALU.max)
                    nc.vector.tensor_mul(t1[:, sl], xf[:, sl], gate[:, sl])

                pool_prods = {}
                for k in range(K2):
                    kh, kw = divmod(k, K)
                    # broadcast dynw row k -> bc [C, N] bf16
                    if k in BCAST_ACT:
                        bc = act_bcast(k)
                    else:
                        bc = bcsp.tile([C, N], bf, tag="bc")
                        nc.sync.dma_start(
                            out=bc[:], in_=ddyn[k:k + 1, :].to_broadcast((C, N)))

                    # product and accumulation. Pool muls are ~4x slower than
                    # DVE muls, so each one's accumulation is deferred by
                    # POOL_INJECT taps -- the strictly-ordered PSUM chain
                    # never waits on a Pool mul, and the injections spread
                    # through the stream instead of piling up at the end.
                    if k in MUL_POOL:
                        prod = bigp.tile([C, N], bf, tag=f"poolprod{k}")
                        nc.gpsimd.tensor_mul(chunks4(prod), win(xpad, k),
                                             chunks4(bc))
                        pool_prods[k] = prod
                    else:
                        prod = prodsp.tile([C, N], bf, tag="prod")
                        nc.vector.tensor_mul(chunks4(prod), win(xpad, k),
                                             chunks4(bc))
                        for ci in range(NCHUNK):
                            accum(ci, fwvT,
                                  prod[:, ci * CH_N:(ci + 1) * CH_N])
                            if k == K2 - 1:
                                emit_bank(ci)
                    if k - POOL_INJECT in pool_prods:
                        pprod = pool_prods.pop(k - POOL_INJECT)
                        for ci in range(NCHUNK):
                            accum(ci, fwvT,
                                  pprod[:, ci * CH_N:(ci + 1) * CH_N])
                    # rank-1 v-bias term before the last taps so the banks
                    # stop right at tap 48's accum
                    if k == 44:
                        for ci in range(NCHUNK):
                            sl2 = slice(ci * CH_N, (ci + 1) * CH_N)
                            accum(ci, fbrow, srow[:, sl2])
                    # gate 1x1 conv + SiLU late in the tap loop: the silu
                    # act-table switch lands in Act's idle window, before the
                    # xf emissions
                    if k == 40:
                        for ci in range(NCHUNK):
                            sl = slice(ci * CH_N, (ci + 1) * CH_N)
                            g_ps = bcpsp.tile([C, 2, 512], f32, tag="bcps")
                            nc.tensor.matmul(
                                g_ps[:, 0, :CH_N], lhsT=wgT,
                                rhs=xpad[:, 3 + ci * 7:10 + ci * 7, 3:3 + W],
                                start=True, stop=True)
                            nc.scalar.activation(gate[:, sl], g_ps[:, 0, :CH_N],
                                                 AF.Silu, bias=gbb, scale=gbs)
                    # S = colsum(dynw) midway (bcps banks free by then)
                    if k == 24:
                        for ci in range(NCHUNK):
                            sl = slice(ci * CH_N, (ci + 1) * CH_N)
                            s_ps = bcpsp.tile([C, 2, 512], f32, tag="bcps")
                            nc.tensor.matmul(s_ps[0:1, 0, :CH_N], lhsT=ones49,
                                             rhs=dynw[:, sl], start=True,
                                             stop=True)
                            nc.scalar.activation(srow[:, sl],
                                                 s_ps[0:1, 0, :CH_N],
                                                 AF.Identity, bias=0.0,
                                                 scale=1.0)

            # ============ channel attention + gate + final combine ============
            with tc.tile_pool(name="mmB", bufs=2, space="PSUM") as mmB:
                stats = workp.tile([C, 2], f32, tag="stats")
                nc.vector.tensor_reduce(out=stats[:, 0:1], in_=asum[:],
                                        axis=AX.X, op=ALU.add)
                nc.vector.tensor_reduce(out=stats[:, 1:2], in_=amax[:],
                                        axis=AX.X, op=ALU.max)
                stat2 = workp.tile([C, 2], f32, tag="stat2")
                if use_collectives:
                    nc.sync.dma_start(out=cc_in_s[:], in_=stats[:, 0:1])
                    nc.sync.dma_start(out=cc_in_m[:], in_=stats[:, 1:2])
                    nc.gpsimd.collective_compute(
                        "AllReduce", ALU.add, replica_groups=groups,
                        ins=[cc_in_s[:]], outs=[cc_out_s[:]])
                    nc.gpsimd.collective_compute(
                        "AllReduce", ALU.max, replica_groups=groups,
                        ins=[cc_in_m[:]], outs=[cc_out_m[:]])
                    nc.sync.dma_start(out=stat2[:, 0:1], in_=cc_out_s[:])
                    nc.sync.dma_start(out=stat2[:, 1:2], in_=cc_out_m[:])
                else:
                    nc.vector.tensor_copy(stat2[:], stats[:])
                nc.scalar.mul(stat2[:, 0:1], stat2[:, 0:1], 1.0 / (H * W))

                r1_ps = mmB.tile([8, 512], f32, tag="mm")
                nc.tensor.matmul(r1_ps[:, :2], lhsT=caw1T, rhs=stat2[:],
                                 start=True, stop=True)
                r1 = workp.tile([8, 2], f32, tag="r1")
                nc.scalar.activation(r1[:], r1_ps[:, :2], AF.Relu, bias=0.0, scale=1.0)
                r2_ps = mmB.tile([C, 512], f32, tag="mm")
                nc.tensor.matmul(r2_ps[:, :2], lhsT=caw2T[:], rhs=r1[:],
                                 start=True, stop=True)
                r2sb = workp.tile([C, 2], f32, tag="r2sb")
                nc.vector.tensor_copy(r2sb[:], r2_ps[:, :2])
                casum = workp.tile([C, 1], f32, tag="casum")
                nc.vector.tensor_add(casum[:], r2sb[:, 0:1], r2sb[:, 1:2])
                # sigmoid(x) = 0.5*(1 + tanh(x/2)); tanh is in the silu act
                # table, so no extra table load at the tail
                cat = workp.tile([C, 1], f32, tag="cat")
                nc.scalar.activation(cat[:], casum[:], AF.Tanh, bias=0.0, scale=0.5)
                ca = workp.tile([C, 1], f32, tag="ca")
                nc.vector.tensor_scalar(ca[:], cat[:], 0.5, 0.5, ALU.mult, ALU.add)

                # out = xid_pre + ca * (gate * xf)   (xid_pre = x_skip*res_scale)
                # chunked by half so the output DMA overlaps the second half
                t2 = bigp.tile([C, N], bf)
                outt = bigp.tile([C, N], bf)
                for hi in range(2):
                    sl = slice(hi * 784, (hi + 1) * 784)
                    nc.vector.tensor_scalar_mul(t2[:, sl], t1[:, sl], ca[:])
                    nc.vector.tensor_add(outt[:, sl], t2[:, sl], xid[:, sl])
                    nc.sync.dma_start(out=d_y[:, sl], in_=outt[:, sl])
            accp_ctx.__exit__(None, None, None)

    nc.compile()
    return nc


def _host_prep(inputs):
    """Build per-core input maps (host work is slicing / dtype casts / tiny
    parameter folding)."""
    f = {k: np.asarray(v, dtype=np.float32) for k, v in inputs.items()}

    # lepe reparam: fold 5 depthwise convs + BNs into one 7x7 kernel + bias
    w7 = f["lk_w"][:, 0] * f["lk_bn_s"][:, None, None]
    w7[:, 1:6, 1:6] += f["dw5"][:, 0] * f["bn5_s"][:, None, None]
    w7[:, 2:5, 2:5] += f["dw3a"][:, 0] * f["bn3a_s"][:, None, None]
    w7[:, 1::2, 1::2] += f["dw3b"][:, 0] * f["bn3b_s"][:, None, None]
    w7[:, ::3, ::3] += f["dw3c"][:, 0] * f["bn3c_s"][:, None, None]
    W_eff = f["lepe_bn_s"][:, None, None] * w7          # [C, 7, 7]
    b_eff = (
        f["lepe_bn_s"]
        * (f["lk_bn_b"] + f["bn5_b"] + f["bn3a_b"] + f["bn3b_b"] + f["bn3c_b"])
        + f["lepe_bn_b"]
    )
    bias_fused = (b_eff + f["fus_b"]).astype(np.float32)
    weff_col = W_eff.reshape(C, K2).astype(np.float32)

    # lepe fp8 DoubleRow weights: pair j -> [2, C] diag values (vertical
    # tap pairs; kh=6 singles leave the second tile zeroed)
    cc = np.arange(C)
    lw8 = np.zeros((C, len(LEPE_PAIRS), 2, C), dtype=FP8)
    for j, (kh_a, kw, kh_b) in enumerate(LEPE_PAIRS):
        lw8[cc, j, 0, cc] = W_eff[:, kh_a, kw].astype(FP8)
        if kh_b is not None:
            lw8[cc, j, 1, cc] = W_eff[:, kh_b, kw].astype(FP8)
    ident = np.eye(C, dtype=BF16)

    fwv = f["fus_w"] @ f["wv"]                 # folded fus @ Wv
    fb = f["fus_w"] @ f["bv"]                  # folded fus @ bv
    est8 = np.zeros((K2, len(BCAST_ACT), 2, C), dtype=FP8)
    for j, k in enumerate(sorted(BCAST_ACT)):
        est8[k, j, 0, :] = 1.0

    # blobs (layouts must match the device-side slicing)
    wk64 = f["wk"] / 64.0                      # device pooled holds 8x8 sums
    m0T = wk64.T @ f["wq"]                     # lhsT of (Wq^T Wk') pooled
    mtb = f["wq"].T @ f["bk"]                  # mt bias = Wq^T bk
    ebv = wk64.T @ f["bq"]                     # eb = SCALE*pooled^T ebv + ebk
    ebk = SCALE * float(f["bk"] @ f["bq"])
    blob_bf = np.concatenate([
        f["gate_w"].T.astype(BF16),            # wgT
        m0T.astype(BF16),                      # m0T
        fwv.T.astype(BF16),                    # fwvT
        mtb[:, None].astype(BF16),             # mt bias col
        ebv[:, None].astype(BF16),             # eb vector col
    ], axis=1)
    blob_f32 = np.concatenate([
        f["ca_w1"].T.astype(np.float32),       # [C, 8]
        np.zeros((C, 1), np.float32),          # (spare)
        bias_fused[:, None],
        f["gate_bn_s"][:, None].astype(np.float32),
        f["gate_bn_b"][:, None].astype(np.float32),
    ], axis=1)
    blob49 = np.concatenate([
        f["ww"].T.astype(BF16),                # wwT [49, 49]
        np.ones((K2, 1), dtype=BF16),          # ones49
        np.full((K2, 1), ebk, dtype=BF16),     # ebk col
    ], axis=1)
    blob1 = np.concatenate([
        np.ones((1, K2), dtype=BF16),          # ones1_49
        f["bw"][None, :].astype(BF16),         # bw row
        fb[None, :].astype(BF16),              # fb row
    ], axis=1)
    common = {
        "lepe_w8": lw8,
        "est8": est8,
        "blob_bf": blob_bf,
        "blob_f32": blob_f32,
        "blob49": blob49,
        "blob1": blob1,
        "ca_w2T": f["ca_w2"].T.astype(np.float32),
    }

    xsk_pad = np.zeros((B, C, H + 6, W + 6), dtype=np.float32)
    xsk_pad[:, :, 3:3 + H, 3:3 + W] = f["x_skip"]
    xid_pre = f["x_skip"] * float(f["res_scale"][0])

    in_maps = []
    for core in range(8):
        b, half = divmod(core, 2)
        r0 = half * HALF
        m = dict(common)
        slab = xsk_pad[b, :, r0:r0 + HP, :]
        m["x_pad"] = slab.astype(BF16)
        m["x_pad8"] = slab.astype(FP8)
        m["x_id"] = xid_pre[b, :, r0:r0 + HALF, :].reshape(C, N).astype(BF16)
        m["x_up"] = f["x_up"][b].astype(BF16)
        in_maps.append(m)
    return in_maps


def kernel(**inputs):
    if "nc" not in _CACHE:
        _CACHE["nc"] = _build_program()
    nc = _CACHE["nc"]
    in_maps = _host_prep(inputs)
    res = run_bass_kernel_spmd(nc, in_maps, list(range(8)))
    out = np.empty((B, C, H, W), dtype=np.float32)
    for core in range(8):
        b, half = divmod(core, 2)
        r0 = half * HALF
        out[b, :, r0:r0 + HALF, :] = np.asarray(
            res.results[core]["y"], dtype=np.float32).reshape(C, HALF, W)
    return out

